# revision 14
# baseline (speedup 1.0000x reference)
"""BasicRGCN Trainium2 kernel — 8-core SPMD Bass/Tile implementation.

Model (PyG-style RGCNConv x2 + global_mean_pool):
  h1 = relu(x @ root1 + b1 + sum_r mean_r(x_src) @ W1[r])
  h2 = relu(h1 @ root2 + b2 + sum_r mean_r(h1_src) @ W2[r])
  out[g] = mean over nodes in graph g of h2            -> [64, 128] f32

Distribution: nodes (and their incoming edges) are sharded over 8 cores by
destination id (12544 nodes/core). Per-relation weights are replicated.

Layer 1: edge features (15-dim x rows, scaled by 1/deg) are pre-gathered on
the host; aggregation uses a one-hot matmul over 32-node dst windows
(128 one-hot columns = 4 relations x 32 offsets).

Layer 2: h1 is exchanged with an AllGather into a replicated table, then
edge rows are fetched with batched SWDGE dma_gather instructions.  Since
gather indices are int16, the table is split into 4 chunks of 25088 rows;
edges are grouped into (64-node dst window, src chunk) cells so each
(tile, chunk) segment is one contiguous gather.  Aggregation uses 256-wide
one-hot matmuls (4 relations x 64 offsets).
"""
import sys
sys.path.insert(0, "/opt/trn_rl_repo")
import numpy as np

import concourse.bass as bass
import concourse.mybir as mybir
import concourse.tile as tile_mod
from concourse.tile import TileContext
from concourse.bacc import Bacc
from concourse.ap import AP
from concourse.masks import make_identity
from concourse.tile_rust import add_dep_helper
from concourse.library_config import mlp

# ---------------------------------------------------------------- constants
NCORES = 8
N = 100000
NPAD = 100352            # 8 * 12544
PC = NPAD // NCORES      # 12544 nodes per core
H = 128                  # hidden dim
F1 = 16                  # padded layer-1 input dim (15 real)
R = 4                    # relations
NGRAPH = 64
NT = 49                  # 256-node output tiles per core

# layer-1 aggregation structure
W = 32                   # dst window width (4W = 128 one-hot columns)
NW = PC // W             # 392 windows per core
TW = 8                   # windows per output tile (256 nodes)

# layer-2 aggregation structure
W2 = 64                  # dst window width (4*W2 = 256 one-hot columns)
NW2 = PC // W2           # 196 windows per core
TW2 = 4                  # windows per output tile (256 nodes)
NCHUNK = 4               # src chunks (int16 gather indices)
CH = NPAD // NCHUNK      # 25088 rows per chunk
GCH2 = 8                 # max groups (128 idx each) per dma_gather

_bf16 = mybir.dt.bfloat16
_f32 = mybir.dt.float32
_fp8 = mybir.dt.float8e4


def _to_bf16(a):
    import ml_dtypes
    return a.astype(ml_dtypes.bfloat16)


# ------------------------------------------------------- tile/walrus patches
def _patch_tile_drain():
    """This deployment's walrus accepts only ONE sync-wait per instruction:
    split the end-of-TileContext drain into single-wait drains."""
    def _patched(self, tick_clock, wait_clock):
        nc = self.nc
        drain_inst = nc.sync.drain()
        wait_clock.add_sem_waits(
            drain_inst.ins, tile_mod.ScopedClock({None: tick_clock.global_clock})
        )
        si = drain_inst.ins.sync_info
        if si is not None and si.on_wait and len(si.on_wait) > 1:
            waits = list(si.on_wait)
            si.on_wait = waits[:1]
            for i in range(1, len(waits)):
                extra = nc.sync.drain()
                esi = extra.ins.sync_info
                if esi is None:
                    extra.ins.sync_info = mybir.SyncInfo(
                        on_wait=[waits[i]], on_update=[])
                else:
                    esi.on_wait = [waits[i]]
        nc.all_engine_barrier()
        assert self.sems is not None
        popped = nc._tile_sem_poison_stack.pop()
        assert popped is self._sem_poison
        nc.clear_and_free_semaphores(list(self.sems.allocated().values()))
        nc.all_engine_barrier()
    TileContext._drain_and_barrier = _patched


_patch_tile_drain()
_legal_ctr = [0]


def _legalize_waits(nc, maxw=1):
    """Split >maxw sync-waits on any instruction onto preceding same-engine
    NoOps (engine streams are in-order, so this is semantics-preserving)."""
    for f in nc.m.functions:
        for blk in f.blocks:
            insts = list(blk.instructions)
            out = []
            changed = False
            for ins in insts:
                si = ins.sync_info
                if si is not None and si.on_wait and len(si.on_wait) > maxw:
                    waits = list(si.on_wait)
                    for i in range(0, len(waits) - maxw, maxw):
                        _legal_ctr[0] += 1
                        nop = mybir.InstNoOp(
                            name=f"legalw-{_legal_ctr[0]}", ins=[], outs=[])
                        nop.engine = ins.engine
                        nop.sync_info = mybir.SyncInfo(
                            on_wait=waits[i:i + maxw], on_update=[])
                        out.append(nop)
                    si.on_wait = waits[len(waits) - maxw:]
                    changed = True
                out.append(ins)
            if changed:
                blk.instructions = out


# ------------------------------------------------------------- host prep
def _host_prep(x, W1, root1, b1, W2w, root2, b2, edge_index, edge_type, batch):
    """Shard/repack all inputs. Returns (per_core_inmaps, host_ctx)."""
    src = np.asarray(edge_index[0], dtype=np.int64)
    dst = np.asarray(edge_index[1], dtype=np.int64)
    rel = np.asarray(edge_type, dtype=np.int64)
    batch = np.asarray(batch, dtype=np.int64)
    x = np.asarray(x, dtype=np.float32)
    E = src.shape[0]

    # per-(relation, dst) in-degree counts -> mean scale
    cnt = np.zeros((R, N), dtype=np.int64)
    np.add.at(cnt, (rel, dst), 1)
    recip = (1.0 / np.maximum(cnt, 1)).astype(np.float32)   # [R, N]

    core_of = dst // PC
    woff = dst % PC

    # ---------------- layer 1 (unchanged structure, host-gathered feed)
    win = woff // W
    key = rel * W + (woff % W)                               # [0, 4W)

    cw = np.zeros((NCORES, NW), dtype=np.int64)
    np.add.at(cw, (core_of, win), 1)
    cap = np.maximum(np.ceil(cw.max(axis=0) / 128).astype(np.int64), 1)  # [NW]
    g_off = np.concatenate([[0], np.cumsum(cap)])            # [NW+1]
    TOTAL_G = int(g_off[-1])

    order = np.lexsort((win, core_of))
    s_src, s_rel, s_dst = src[order], rel[order], dst[order]
    s_core, s_win = core_of[order], win[order]
    s_key = key[order]
    s_scale = recip[s_rel, s_dst].astype(np.float32)

    NSLOT = TOTAL_G * 128
    keys_all = np.full((NCORES, NSLOT), -1.0, dtype=np.float32)
    xsl_all = np.zeros((NCORES, NSLOT, F1), dtype=np.float32)

    cell_id = s_core * NW + s_win
    cell_start = np.zeros(NCORES * NW + 1, dtype=np.int64)
    np.add.at(cell_start, cell_id + 1, 1)
    cell_start = np.cumsum(cell_start)
    pos_in_cell = np.arange(E) - cell_start[cell_id]
    slot = (g_off[s_win] * 128 + pos_in_cell).astype(np.int64)

    keys_all[s_core, slot] = s_key
    xsl_all[s_core, slot, :15] = x[s_src] * s_scale[:, None]

    keys_pg = _to_bf16(
        keys_all.reshape(NCORES, TOTAL_G, 128).transpose(0, 2, 1).copy())
    xsl_pg = _to_bf16(
        xsl_all.reshape(NCORES, TOTAL_G, 128, F1).transpose(0, 2, 1, 3).copy())

    # ---------------- layer 2 (chunked dma_gather structure)
    win2 = woff // W2
    key2 = rel * W2 + (woff % W2)                            # [0, 4*W2)
    chnk = src // CH

    cnt2 = np.zeros((NCORES, NW2, NCHUNK), dtype=np.int64)
    np.add.at(cnt2, (core_of, win2, chnk), 1)
    cap2 = np.ceil(cnt2.max(axis=0) / 128).astype(np.int64)  # [NW2, NCHUNK]
    cap2[:, 0] = np.maximum(cap2[:, 0], 1)

    # group layout: per tile t, chunk-major segments of its TW2 windows
    group_base = np.zeros((NW2, NCHUNK), dtype=np.int64)
    tile_g0 = np.zeros(NT + 1, dtype=np.int64)
    seg_off = np.zeros((NT, NCHUNK + 1), dtype=np.int64)
    g = 0
    for t in range(NT):
        tile_g0[t] = g
        for c in range(NCHUNK):
            seg_off[t][c] = g
            for wi in range(TW2):
                w = t * TW2 + wi
                group_base[w, c] = g
                g += cap2[w, c]
        seg_off[t][NCHUNK] = g
    tile_g0[NT] = g
    TOTAL_G2 = int(g)

    order2 = np.lexsort((chnk, win2, core_of))
    s2_src, s2_rel = src[order2], rel[order2]
    s2_core, s2_win, s2_chnk = core_of[order2], win2[order2], chnk[order2]
    s2_key = key2[order2]

    cell_id2 = (s2_core * NW2 + s2_win) * NCHUNK + s2_chnk
    cell_start2 = np.zeros(NCORES * NW2 * NCHUNK + 1, dtype=np.int64)
    np.add.at(cell_start2, cell_id2 + 1, 1)
    cell_start2 = np.cumsum(cell_start2)
    pos2 = np.arange(E) - cell_start2[cell_id2]
    slot2 = ((group_base[s2_win, s2_chnk] + pos2 // 128) * 128
             + pos2 % 128).astype(np.int64)

    NSLOT2 = TOTAL_G2 * 128
    keys2_all = np.full((NCORES, NSLOT2), -1.0, dtype=np.float32)
    idx16_all = np.zeros((NCORES, NSLOT2), dtype=np.int16)
    keys2_all[s2_core, slot2] = s2_key
    idx16_all[s2_core, slot2] = (s2_src - s2_chnk * CH).astype(np.int16)

    keys2_pg = _to_bf16(
        keys2_all.reshape(NCORES, TOTAL_G2, 128).transpose(0, 2, 1).copy())
    # wrapped idx layout: element i at [i % 16, i // 16], replicated to 128
    idxw = idx16_all.reshape(NCORES, NSLOT2 // 16, 16).transpose(0, 2, 1)
    idxw = np.tile(idxw, (1, 8, 1)).copy()                   # [NC,128,NSLOT2/16]

    # scale columns for layer 2: col (w2, r*W2+o)
    sc2 = np.zeros((NCORES, NW2 * 4 * W2), dtype=np.float32)
    r_grid = np.repeat(np.arange(R), W2)
    for k in range(NCORES):
        nodes_w = (k * PC + (np.arange(NW2)[:, None] * W2
                             + np.tile(np.arange(W2), R)[None, :]))
        nodes_w = np.minimum(nodes_w, N - 1)
        sc2[k] = recip[r_grid[None, :], nodes_w].reshape(-1)
    sc2_bf = _to_bf16(sc2)
    sc2_rep = np.broadcast_to(
        sc2_bf[:, None, :], (NCORES, 128, NW2 * 4 * W2)).copy()

    # batch one-hot [PC, 64] per core
    bone = np.zeros((NCORES, PC, NGRAPH), dtype=np.float32)
    for k in range(NCORES):
        nd = k * PC + np.arange(PC)
        real = nd < N
        bone[k, real, batch[nd[real]]] = 1.0
    bone_bf = _to_bf16(bone)

    # x^T own block, padded feat rows [128, PC]
    xT = np.zeros((NCORES, 128, PC), dtype=np.float32)
    for k in range(NCORES):
        nd = k * PC + np.arange(PC)
        real = nd < N
        xT[k][:15][:, real] = x[nd[real]].T
    xT_bf = _to_bf16(xT)

    # weights (replicated)
    def padw(w, rows):
        out = np.zeros((128, H), dtype=np.float32)
        out[:rows] = w
        return _to_bf16(out)

    W1p = np.stack([padw(np.asarray(W1)[r], 15) for r in range(R)])
    root1p = padw(np.asarray(root1), 15)
    W2p = np.stack([padw(np.asarray(W2w)[r], H) for r in range(R)])
    root2p = padw(np.asarray(root2), H)
    b1f = np.asarray(b1, dtype=np.float32).reshape(H, 1)
    b2f = np.asarray(b2, dtype=np.float32).reshape(H, 1)

    in_maps = []
    for k in range(NCORES):
        in_maps.append({
            "xsl": xsl_pg[k],          # [128, TOTAL_G, F1] bf16
            "keys": keys_pg[k],        # [128, TOTAL_G] bf16
            "keys2": keys2_pg[k],      # [128, TOTAL_G2] bf16
            "idx2": idxw[k],           # [128, NSLOT2/16] int16
            "screp2": sc2_rep[k],      # [128, NW2*256] bf16
            "bone": bone_bf[k],        # [PC, 64] bf16
            "xT": xT_bf[k],            # [128, PC] bf16
            "W1p": W1p, "root1p": root1p,
            "W2p": W2p, "root2p": root2p,
            "b1": b1f, "b2": b2f,
        })

    gcounts = np.maximum(np.bincount(batch, minlength=NGRAPH), 1).astype(np.float32)
    host_ctx = {"cap": cap, "g_off": g_off, "TOTAL_G": TOTAL_G,
                "cap2": cap2, "group_base": group_base,
                "tile_g0": tile_g0, "seg_off": seg_off,
                "TOTAL_G2": TOTAL_G2, "gcounts": gcounts}
    return in_maps, host_ctx


# ------------------------------------------------------------- device build
def _build_nc(hc, legalize=True):
    cap, g_off, TOTAL_G = hc["cap"], hc["g_off"], hc["TOTAL_G"]
    cap2, group_base = hc["cap2"], hc["group_base"]
    tile_g0, seg_off, TOTAL_G2 = hc["tile_g0"], hc["seg_off"], hc["TOTAL_G2"]

    GT1 = max(int(g_off[(t + 1) * TW] - g_off[t * TW]) for t in range(NT))
    GT2 = max(int(tile_g0[t + 1] - tile_g0[t]) for t in range(NT))

    nc = Bacc("TRN2", num_devices=NCORES, num_swdge_queues=4)
    xsl = nc.dram_tensor("xsl", [128, TOTAL_G, F1], _bf16, kind="ExternalInput")
    keys = nc.dram_tensor("keys", [128, TOTAL_G], _bf16, kind="ExternalInput")
    keys2 = nc.dram_tensor("keys2", [128, TOTAL_G2], _bf16, kind="ExternalInput")
    idx2 = nc.dram_tensor("idx2", [128, TOTAL_G2 * 8], mybir.dt.int16,
                          kind="ExternalInput")
    screp2 = nc.dram_tensor("screp2", [128, NW2 * 4 * W2], _bf16,
                            kind="ExternalInput")
    bone = nc.dram_tensor("bone", [PC, NGRAPH], _bf16, kind="ExternalInput")
    xT = nc.dram_tensor("xT", [128, PC], _bf16, kind="ExternalInput")
    W1p = nc.dram_tensor("W1p", [R, 128, H], _bf16, kind="ExternalInput")
    root1p = nc.dram_tensor("root1p", [128, H], _bf16, kind="ExternalInput")
    W2p = nc.dram_tensor("W2p", [R, 128, H], _bf16, kind="ExternalInput")
    root2p = nc.dram_tensor("root2p", [128, H], _bf16, kind="ExternalInput")
    b1 = nc.dram_tensor("b1", [H, 1], _f32, kind="ExternalInput")
    b2 = nc.dram_tensor("b2", [H, 1], _f32, kind="ExternalInput")
    h1own = nc.dram_tensor("h1own", [PC, H], _bf16, kind="Internal")
    h1tab = nc.dram_tensor("h1tab", [NPAD, H], _bf16, kind="Internal",
                           addr_space="Shared")
    pool_out = nc.dram_tensor("pool_out", [NGRAPH, H], _f32, kind="ExternalOutput")

    with TileContext(nc, num_cores=NCORES) as tc:
        import contextlib
        with contextlib.ExitStack() as ctx:
            const_p = ctx.enter_context(tc.tile_pool(name="const", bufs=1))
            wpool = ctx.enter_context(tc.tile_pool(name="wts", bufs=1))
            hpool = ctx.enter_context(tc.tile_pool(name="hT", bufs=1))
            feed_p = ctx.enter_context(tc.tile_pool(name="feed", bufs=3))
            oh_p = ctx.enter_context(tc.tile_pool(name="oh", bufs=2))
            g_p = ctx.enter_context(tc.tile_pool(name="gat", bufs=2))
            idx_p = ctx.enter_context(tc.tile_pool(name="idxp", bufs=3))
            sb_p = ctx.enter_context(tc.tile_pool(name="stile", bufs=2))
            sc_p = ctx.enter_context(tc.tile_pool(name="sctile", bufs=2))
            tok_p = ctx.enter_context(tc.tile_pool(name="tok", bufs=3))
            ho_p = ctx.enter_context(tc.tile_pool(name="hop", bufs=2))
            bo_p = ctx.enter_context(tc.tile_pool(name="bo", bufs=3))
            misc_p = ctx.enter_context(tc.tile_pool(name="misc", bufs=2))
            ps_agg = ctx.enter_context(
                tc.tile_pool(name="ps_agg", bufs=2, space="PSUM"))
            ps_out = ctx.enter_context(
                tc.tile_pool(name="ps_out", bufs=1, space="PSUM"))
            ps_tr = ctx.enter_context(
                tc.tile_pool(name="ps_tr", bufs=2, space="PSUM"))
            ps_pool = ctx.enter_context(
                tc.tile_pool(name="ps_pool", bufs=1, space="PSUM"))

            nc.gpsimd.load_library(mlp)

            # iota constants (one period; broadcast via 0-step AP in is_equal)
            iota_i1 = const_p.tile([128, 128], mybir.dt.int32, tag="ioi1")
            nc.gpsimd.iota(iota_i1[:], pattern=[[1, 128]], base=0,
                           channel_multiplier=0)
            iota1 = const_p.tile([128, 128], _bf16, tag="io1")
            nc.vector.tensor_copy(iota1[:], iota_i1[:])
            iota_i2 = const_p.tile([128, 4 * W2], mybir.dt.int32, tag="ioi2")
            nc.gpsimd.iota(iota_i2[:], pattern=[[1, 4 * W2]], base=0,
                           channel_multiplier=0)
            iota2 = const_p.tile([128, 4 * W2], _bf16, tag="io2")
            nc.vector.tensor_copy(iota2[:], iota_i2[:])
            ident = const_p.tile([128, 128], _bf16, tag="ident")
            make_identity(nc, ident[:])

            # weights resident in SBUF
            w1t = [wpool.tile([128, H], _bf16, tag=f"w1_{r}", name=f"w1_{r}")
                   for r in range(R)]
            w2t = [wpool.tile([128, H], _bf16, tag=f"w2_{r}", name=f"w2_{r}")
                   for r in range(R)]
            r1t = wpool.tile([128, H], _bf16, tag="r1")
            r2t = wpool.tile([128, H], _bf16, tag="r2")
            b1t = wpool.tile([H, 1], _f32, tag="b1")
            b2t = wpool.tile([H, 1], _f32, tag="b2")
            for r in range(R):
                nc.sync.dma_start(out=w1t[r][:], in_=W1p[r])
                nc.sync.dma_start(out=w2t[r][:], in_=W2p[r])
            nc.sync.dma_start(out=r1t[:], in_=root1p[:, :])
            nc.sync.dma_start(out=r2t[:], in_=root2p[:, :])
            nc.sync.dma_start(out=b1t[:], in_=b1[:, :])
            nc.sync.dma_start(out=b2t[:], in_=b2[:, :])

            hT_x = hpool.tile([128, PC], _bf16, tag="hT_x")     # layer1 rhs
            hT_1 = hpool.tile([128, PC], _bf16, tag="hT_1")     # layer1 out
            nc.sync.dma_start(out=hT_x[:], in_=xT[:, :])

            pool_acc = const_p.tile([NGRAPH, H], _f32, tag="pacc")
            nc.vector.memset(pool_acc[:], 0.0)

            def bcast_inner(tile_ap, ncols, inner):
                """[128, ncols] -> AP [128, ncols, inner] (step-0 inner)."""
                base = tile_ap
                newap = [list(base.ap[0]), [base.ap[-1][0], ncols], [0, inner]]
                return AP(base.tensor, base.offset, newap)

            def bcast_outer(tile_ap, ng, inner):
                """[128, inner] -> AP [128, ng, inner] (step-0 middle)."""
                base = tile_ap
                newap = [list(base.ap[0]), [0, ng], [base.ap[-1][0], inner]]
                return AP(base.tensor, base.offset, newap)

            def tail(L, t, op_ps, hT_out_full):
                """bias+relu, transpose to token-major, h1own store / pool."""
                nsl = slice(t * 256, (t + 1) * 256)
                if hT_out_full is not None:
                    hO = hT_out_full[:, nsl]
                else:
                    hO = ho_p.tile([128, 256], _bf16, tag="hO", name="hO")
                nc.scalar.activation(
                    out=hO[:, :] if hT_out_full is None else hO,
                    in_=op_ps[:, :],
                    func=mybir.ActivationFunctionType.Relu,
                    bias=b1t[:] if L == 1 else b2t[:], scale=1.0)
                for half in range(2):
                    hsl = slice(half * 128, (half + 1) * 128)
                    tr = ps_tr.tile([128, 128], _bf16, tag="tr")
                    nc.tensor.transpose(tr[:, :], hO[:, hsl], ident[:])
                    tok = tok_p.tile([128, 128], _bf16, tag="tok")
                    nc.scalar.activation(
                        out=tok[:], in_=tr[:, :],
                        func=mybir.ActivationFunctionType.Copy)
                    row0 = t * 256 + half * 128
                    if L == 1:
                        nc.sync.dma_start(
                            out=h1own[row0:row0 + 128, :], in_=tok[:])
                    else:
                        bt_t = bo_p.tile([128, NGRAPH], _bf16, tag="bt")
                        nc.sync.dma_start(
                            out=bt_t[:], in_=bone[row0:row0 + 128, :])
                        pp = ps_pool.tile([NGRAPH, H], _f32, tag="pp")
                        nc.tensor.matmul(pp[:, :], lhsT=bt_t[:],
                                         rhs=tok[:], start=True, stop=True)
                        nc.vector.tensor_tensor(
                            out=pool_acc[:], in0=pool_acc[:], in1=pp[:, :],
                            op=mybir.AluOpType.add)

            # ---------------------------------------------------- layer 1
            for t in range(NT):
                w0 = t * TW
                g0, g1 = int(g_off[w0]), int(g_off[w0 + TW])
                ng = g1 - g0
                feedt = feed_p.tile([128, GT1, F1], _bf16, tag="feed")
                nc.sync.dma_start(out=feedt[:, :ng, :], in_=xsl[:, g0:g1, :])
                keyt = misc_p.tile([128, GT1], _bf16, tag="keyt")
                nc.sync.dma_start(out=keyt[:, :ng], in_=keys[:, g0:g1])
                oht = oh_p.tile([128, GT1 * 128], _bf16, tag="oht")
                nc.vector.tensor_tensor(
                    out=oht[:, :ng * 128],
                    in0=bcast_outer(iota1[:, :], ng, 128),
                    in1=bcast_inner(keyt[:, :ng], ng, 128),
                    op=mybir.AluOpType.is_equal)

                agg = [ps_agg.tile([128, 512], _f32, tag=f"agg{h}",
                                   name=f"agg{h}") for h in range(2)]
                for wi in range(TW):
                    w = w0 + wi
                    colsl = slice((wi % 4) * 128, (wi % 4) * 128 + 128)
                    ngw = int(cap[w])
                    for j in range(ngw):
                        gg = int(g_off[w]) + j - g0
                        nc.tensor.matmul(
                            agg[wi // 4][:F1, colsl],
                            lhsT=feedt[:, gg, :],
                            rhs=oht[:, gg * 128:(gg + 1) * 128],
                            start=(j == 0), stop=(j == ngw - 1))

                st = sb_p.tile([128, TW * 128], _bf16, tag="st")
                for h in range(2):
                    nc.vector.tensor_copy(
                        st[:F1, h * 512:(h + 1) * 512], agg[h][:F1, :])

                op_ps = ps_out.tile([128, 256], _f32, tag="ops")
                nsl = slice(t * 256, (t + 1) * 256)
                nc.tensor.matmul(op_ps[:, :], lhsT=r1t[:F1, :],
                                 rhs=hT_x[:F1, nsl], start=True, stop=False)
                st3 = st[:F1, :].rearrange("p (a b) -> p a b", b=128)
                for r in range(R):
                    nc.tensor.matmul(op_ps[:, :], lhsT=w1t[r][:F1, :],
                                     rhs=st3[:, :, r * W:(r + 1) * W],
                                     start=False, stop=(r == R - 1))
                tail(1, t, op_ps, hT_1)

            # ------------------------------------------------- all-gather
            ag = nc.gpsimd.collective_compute(
                "AllGather", mybir.AluOpType.bypass,
                replica_groups=[list(range(NCORES))],
                ins=[h1own[:, :]], outs=[h1tab[:, :]])
            ag_inst = ag.ins if hasattr(ag, "ins") else ag
            ag_dep_done = [False]
            qrr = [0]

            # ---------------------------------------------------- layer 2
            for t in range(NT):
                tg0, tg1 = int(tile_g0[t]), int(tile_g0[t + 1])
                gt_n = tg1 - tg0
                idxt = idx_p.tile([128, GT2 * 8], mybir.dt.int16, tag="idxt")
                nc.sync.dma_start(out=idxt[:, :gt_n * 8],
                                  in_=idx2[:, tg0 * 8:tg1 * 8])
                keyt2 = misc_p.tile([128, GT2], _bf16, tag="keyt2")
                nc.sync.dma_start(out=keyt2[:, :gt_n], in_=keys2[:, tg0:tg1])

                gtb = g_p.tile([128, GT2, H], _bf16, tag="gtb")
                for c in range(NCHUNK):
                    s0 = int(seg_off[t][c]) - tg0
                    s1 = int(seg_off[t][c + 1]) - tg0
                    for b0 in range(s0, s1, GCH2):
                        b1_ = min(b0 + GCH2, s1)
                        nb = b1_ - b0
                        gi = nc.gpsimd.dma_gather(
                            gtb[:, b0:b1_, :],
                            h1tab[c * CH:(c + 1) * CH, :],
                            idxt[:, b0 * 8:b1_ * 8],
                            nb * 128, nb * 128, H,
                            queue_num=qrr[0] % 4)
                        qrr[0] += 1
                        if not ag_dep_done[0]:
                            add_dep_helper(
                                gi.ins, ag_inst,
                                reason="L2 gather reads AllGather output")
                            ag_dep_done[0] = True

                oht2 = oh_p.tile([128, GT2 * 4 * W2], _bf16, tag="oht2")
                nc.vector.tensor_tensor(
                    out=oht2[:, :gt_n * 4 * W2],
                    in0=bcast_outer(iota2[:, :], gt_n, 4 * W2),
                    in1=bcast_inner(keyt2[:, :gt_n], gt_n, 4 * W2),
                    op=mybir.AluOpType.is_equal)

                agg = [ps_agg.tile([128, 512], _f32, tag=f"agg{h}",
                                   name=f"agg{h}") for h in range(2)]
                for wi in range(TW2):
                    w = t * TW2 + wi
                    colsl = slice((wi % 2) * 256, (wi % 2) * 256 + 256)
                    seq = [(c, j) for c in range(NCHUNK)
                           for j in range(int(cap2[w, c]))]
                    for si, (c, j) in enumerate(seq):
                        gg = int(group_base[w, c]) + j - tg0
                        nc.tensor.matmul(
                            agg[wi // 2][:, colsl],
                            lhsT=gtb[:, gg, :],
                            rhs=oht2[:, gg * 256:(gg + 1) * 256],
                            start=(si == 0), stop=(si == len(seq) - 1))

                st = sb_p.tile([128, TW2 * 256], _bf16, tag="st")
                sct = sc_p.tile([128, TW2 * 256], _bf16, tag="sct")
                nc.sync.dma_start(
                    out=sct[:], in_=screp2[:, t * 1024:(t + 1) * 1024])
                for h in range(2):
                    nc.vector.tensor_tensor(
                        out=st[:, h * 512:(h + 1) * 512],
                        in0=agg[h][:, :], in1=sct[:, h * 512:(h + 1) * 512],
                        op=mybir.AluOpType.mult)

                op_ps = ps_out.tile([128, 256], _f32, tag="ops")
                nsl = slice(t * 256, (t + 1) * 256)
                nc.tensor.matmul(op_ps[:, :], lhsT=r2t[:, :],
                                 rhs=hT_1[:, nsl], start=True, stop=False)
                st3 = st[:, :].rearrange("p (a b) -> p a b", b=256)
                for r in range(R):
                    nc.tensor.matmul(op_ps[:, :], lhsT=w2t[r][:, :],
                                     rhs=st3[:, :, r * W2:(r + 1) * W2],
                                     start=False, stop=(r == R - 1))
                tail(2, t, op_ps, None)

            nc.sync.dma_start(out=pool_out[:, :], in_=pool_acc[:])

    nc.finalize()
    if legalize:
        _legalize_waits(nc)
    return nc


# ------------------------------------------------------------- runner
_CACHE = {}


def _get_compiled(hc):
    key = ("nc", hc["TOTAL_G"], hc["TOTAL_G2"],
           tuple(hc["cap"].tolist()),
           tuple(hc["cap2"].reshape(-1).tolist()))
    if key not in _CACHE:
        import jax
        from jax.sharding import Mesh, PartitionSpec
        from jax.experimental.shard_map import shard_map
        from concourse.bass2jax import (
            _bass_exec_p, partition_id_tensor, install_neuronx_cc_hook)
        install_neuronx_cc_hook()
        nc = _build_nc(hc)

        partition_name = (nc.partition_id_tensor.name
                          if nc.partition_id_tensor else None)
        in_names, out_names, out_avals = [], [], []
        for alloc in nc.m.functions[0].allocations:
            if not isinstance(alloc, mybir.MemoryLocationSet):
                continue
            name = alloc.memorylocations[0].name
            if alloc.kind == "ExternalInput":
                if name != partition_name and name != (
                        nc.dbg_addr.name if nc.dbg_addr is not None else None):
                    in_names.append(name)
            elif alloc.kind == "ExternalOutput":
                out_names.append(name)
                out_avals.append(jax.core.ShapedArray(
                    tuple(alloc.tensor_shape), mybir.dt.np(alloc.dtype)))
        n_params, n_outs = len(in_names), len(out_names)
        all_in = list(in_names) + list(out_names)
        if nc.dbg_addr is not None:
            all_in.append(nc.dbg_addr.name)
        if partition_name is not None:
            all_in.append(partition_name)

        def _body(*args):
            operands = list(args)
            if nc.dbg_addr is not None:
                operands.append(jax.numpy.zeros((1, 2), jax.numpy.uint32))
            if partition_name is not None:
                operands.append(partition_id_tensor())
            outs = _bass_exec_p.bind(
                *operands, out_avals=tuple(out_avals),
                in_names=tuple(all_in), out_names=tuple(out_names),
                lowering_input_output_aliases=(),
                sim_require_finite=False, sim_require_nnan=False, nc=nc)
            return tuple(outs)

        devices = jax.devices()[:NCORES]
        mesh = Mesh(np.asarray(devices), ("core",))
        sharded = jax.jit(
            shard_map(_body, mesh=mesh,
                      in_specs=(PartitionSpec("core"),) * (n_params + n_outs),
                      out_specs=(PartitionSpec("core"),) * n_outs,
                      check_rep=False),
            donate_argnums=tuple(range(n_params, n_params + n_outs)),
            keep_unused=True)
        _CACHE[key] = (sharded, in_names, out_names, out_avals, mesh)
    return _CACHE[key]


def run_device(in_maps, hc):
    import jax
    sharded, in_names, out_names, out_avals, mesh = _get_compiled(hc)
    concat_in = [
        np.concatenate([np.asarray(in_maps[c][name]) for c in range(NCORES)],
                       axis=0)
        for name in in_names]
    concat_zeros = [
        np.zeros((NCORES * a.shape[0], *a.shape[1:]), a.dtype)
        for a in out_avals]
    out_arrs = sharded(*concat_in, *concat_zeros)
    jax.block_until_ready(out_arrs)
    res = [
        {name: np.asarray(out_arrs[i]).reshape(NCORES, *out_avals[i].shape)[c]
         for i, name in enumerate(out_names)}
        for c in range(NCORES)]
    return res


def kernel(x, W1, root1, b1, W2, root2, b2, edge_index, edge_type, batch):
    in_maps, hc = _host_prep(x, W1, root1, b1, W2, root2, b2,
                             edge_index, edge_type, batch)
    res = run_device(in_maps, hc)
    total = np.zeros((NGRAPH, H), dtype=np.float32)
    for k in range(NCORES):
        total += res[k]["pool_out"]
    return (total / hc["gcounts"][:, None]).astype(np.float32)


# revision 15
# speedup vs baseline: 1.1288x; 1.1288x over previous
"""BasicRGCN Trainium2 kernel — 8-core SPMD Bass/Tile implementation.

Model (PyG-style RGCNConv x2 + global_mean_pool):
  h1 = relu(x @ root1 + b1 + sum_r mean_r(x_src) @ W1[r])
  h2 = relu(h1 @ root2 + b2 + sum_r mean_r(h1_src) @ W2[r])
  out[g] = mean over nodes in graph g of h2            -> [64, 128] f32

Distribution: nodes (and their incoming edges) are sharded over 8 cores by
destination id (12544 nodes/core). Per-relation weights are replicated.

Layer 1: edge features (15-dim x rows, scaled by 1/deg) are pre-gathered on
the host; aggregation uses a one-hot matmul over 32-node dst windows
(128 one-hot columns = 4 relations x 32 offsets).

Layer 2: h1 is exchanged with an AllGather into a replicated table, then
edge rows are fetched with batched SWDGE dma_gather instructions.  Since
gather indices are int16, the table is split into 4 chunks of 25088 rows;
edges are grouped into (64-node dst window, src chunk) cells so each
(tile, chunk) segment is one contiguous gather.  Aggregation uses 256-wide
one-hot matmuls (4 relations x 64 offsets).
"""
import sys
sys.path.insert(0, "/opt/trn_rl_repo")
import numpy as np

import concourse.bass as bass
import concourse.mybir as mybir
import concourse.tile as tile_mod
from concourse.tile import TileContext
from concourse.bacc import Bacc
from concourse.ap import AP
from concourse.masks import make_identity
from concourse.tile_rust import add_dep_helper
from concourse.library_config import mlp

# ---------------------------------------------------------------- constants
NCORES = 8
N = 100000
NPAD = 100352            # 8 * 12544
PC = NPAD // NCORES      # 12544 nodes per core
H = 128                  # hidden dim
F1 = 16                  # padded layer-1 input dim (15 real)
R = 4                    # relations
NGRAPH = 64
NT = 49                  # 256-node output tiles per core

# layer-1 aggregation structure
W = 32                   # dst window width (4W = 128 one-hot columns)
NW = PC // W             # 392 windows per core
TW = 8                   # windows per output tile (256 nodes)

# layer-2 aggregation structure
W2 = 64                  # dst window width (4*W2 = 256 one-hot columns)
NW2 = PC // W2           # 196 windows per core
TW2 = 4                  # windows per output tile (256 nodes)
NCHUNK = 4               # src chunks (int16 gather indices)
CH = NPAD // NCHUNK      # 25088 rows per chunk
GCH2 = 8                 # max groups (128 idx each) per dma_gather

_bf16 = mybir.dt.bfloat16
_f32 = mybir.dt.float32
_fp8 = mybir.dt.float8e4


def _to_bf16(a):
    import ml_dtypes
    return a.astype(ml_dtypes.bfloat16)


# ------------------------------------------------------- tile/walrus patches
def _patch_tile_drain():
    """This deployment's walrus accepts only ONE sync-wait per instruction:
    split the end-of-TileContext drain into single-wait drains."""
    def _patched(self, tick_clock, wait_clock):
        nc = self.nc
        drain_inst = nc.sync.drain()
        wait_clock.add_sem_waits(
            drain_inst.ins, tile_mod.ScopedClock({None: tick_clock.global_clock})
        )
        si = drain_inst.ins.sync_info
        if si is not None and si.on_wait and len(si.on_wait) > 1:
            waits = list(si.on_wait)
            si.on_wait = waits[:1]
            for i in range(1, len(waits)):
                extra = nc.sync.drain()
                esi = extra.ins.sync_info
                if esi is None:
                    extra.ins.sync_info = mybir.SyncInfo(
                        on_wait=[waits[i]], on_update=[])
                else:
                    esi.on_wait = [waits[i]]
        nc.all_engine_barrier()
        assert self.sems is not None
        popped = nc._tile_sem_poison_stack.pop()
        assert popped is self._sem_poison
        nc.clear_and_free_semaphores(list(self.sems.allocated().values()))
        nc.all_engine_barrier()
    TileContext._drain_and_barrier = _patched


_patch_tile_drain()
_legal_ctr = [0]


def _legalize_waits(nc, maxw=1):
    """Split >maxw sync-waits on any instruction onto preceding same-engine
    NoOps (engine streams are in-order, so this is semantics-preserving)."""
    for f in nc.m.functions:
        for blk in f.blocks:
            insts = list(blk.instructions)
            out = []
            changed = False
            for ins in insts:
                si = ins.sync_info
                if si is not None and si.on_wait and len(si.on_wait) > maxw:
                    waits = list(si.on_wait)
                    for i in range(0, len(waits) - maxw, maxw):
                        _legal_ctr[0] += 1
                        nop = mybir.InstNoOp(
                            name=f"legalw-{_legal_ctr[0]}", ins=[], outs=[])
                        nop.engine = ins.engine
                        nop.sync_info = mybir.SyncInfo(
                            on_wait=waits[i:i + maxw], on_update=[])
                        out.append(nop)
                    si.on_wait = waits[len(waits) - maxw:]
                    changed = True
                out.append(ins)
            if changed:
                blk.instructions = out


# ------------------------------------------------------------- host prep
def _host_prep(x, W1, root1, b1, W2w, root2, b2, edge_index, edge_type, batch):
    """Shard/repack all inputs. Returns (per_core_inmaps, host_ctx)."""
    src = np.asarray(edge_index[0], dtype=np.int64)
    dst = np.asarray(edge_index[1], dtype=np.int64)
    rel = np.asarray(edge_type, dtype=np.int64)
    batch = np.asarray(batch, dtype=np.int64)
    x = np.asarray(x, dtype=np.float32)
    E = src.shape[0]

    # per-(relation, dst) in-degree counts -> mean scale
    cnt = np.zeros((R, N), dtype=np.int64)
    np.add.at(cnt, (rel, dst), 1)
    recip = (1.0 / np.maximum(cnt, 1)).astype(np.float32)   # [R, N]

    core_of = dst // PC
    woff = dst % PC

    # ---------------- layer 1 (unchanged structure, host-gathered feed)
    win = woff // W
    key = rel * W + (woff % W)                               # [0, 4W)

    cw = np.zeros((NCORES, NW), dtype=np.int64)
    np.add.at(cw, (core_of, win), 1)
    cap = np.maximum(np.ceil(cw.max(axis=0) / 128).astype(np.int64), 1)  # [NW]
    g_off = np.concatenate([[0], np.cumsum(cap)])            # [NW+1]
    TOTAL_G = int(g_off[-1])

    order = np.lexsort((win, core_of))
    s_src, s_rel, s_dst = src[order], rel[order], dst[order]
    s_core, s_win = core_of[order], win[order]
    s_key = key[order]
    s_scale = recip[s_rel, s_dst].astype(np.float32)

    NSLOT = TOTAL_G * 128
    keys_all = np.full((NCORES, NSLOT), -1.0, dtype=np.float32)
    xsl_all = np.zeros((NCORES, NSLOT, F1), dtype=np.float32)

    cell_id = s_core * NW + s_win
    cell_start = np.zeros(NCORES * NW + 1, dtype=np.int64)
    np.add.at(cell_start, cell_id + 1, 1)
    cell_start = np.cumsum(cell_start)
    pos_in_cell = np.arange(E) - cell_start[cell_id]
    slot = (g_off[s_win] * 128 + pos_in_cell).astype(np.int64)

    keys_all[s_core, slot] = s_key
    xsl_all[s_core, slot, :15] = x[s_src] * s_scale[:, None]

    keys_pg = _to_bf16(
        keys_all.reshape(NCORES, TOTAL_G, 128).transpose(0, 2, 1).copy())
    xsl_pg = _to_bf16(
        xsl_all.reshape(NCORES, TOTAL_G, 128, F1).transpose(0, 2, 1, 3).copy())

    # ---------------- layer 2 (chunked dma_gather structure)
    win2 = woff // W2
    key2 = rel * W2 + (woff % W2)                            # [0, 4*W2)
    chnk = src // CH

    cnt2 = np.zeros((NCORES, NW2, NCHUNK), dtype=np.int64)
    np.add.at(cnt2, (core_of, win2, chnk), 1)
    cap2 = np.ceil(cnt2.max(axis=0) / 128).astype(np.int64)  # [NW2, NCHUNK]
    cap2[:, 0] = np.maximum(cap2[:, 0], 1)

    # group layout: per tile t, chunk-major segments of its TW2 windows
    group_base = np.zeros((NW2, NCHUNK), dtype=np.int64)
    tile_g0 = np.zeros(NT + 1, dtype=np.int64)
    seg_off = np.zeros((NT, NCHUNK + 1), dtype=np.int64)
    g = 0
    for t in range(NT):
        tile_g0[t] = g
        for c in range(NCHUNK):
            seg_off[t][c] = g
            for wi in range(TW2):
                w = t * TW2 + wi
                group_base[w, c] = g
                g += cap2[w, c]
        seg_off[t][NCHUNK] = g
    tile_g0[NT] = g
    TOTAL_G2 = int(g)

    order2 = np.lexsort((chnk, win2, core_of))
    s2_src, s2_rel = src[order2], rel[order2]
    s2_core, s2_win, s2_chnk = core_of[order2], win2[order2], chnk[order2]
    s2_key = key2[order2]

    cell_id2 = (s2_core * NW2 + s2_win) * NCHUNK + s2_chnk
    cell_start2 = np.zeros(NCORES * NW2 * NCHUNK + 1, dtype=np.int64)
    np.add.at(cell_start2, cell_id2 + 1, 1)
    cell_start2 = np.cumsum(cell_start2)
    pos2 = np.arange(E) - cell_start2[cell_id2]
    slot2 = ((group_base[s2_win, s2_chnk] + pos2 // 128) * 128
             + pos2 % 128).astype(np.int64)

    NSLOT2 = TOTAL_G2 * 128
    keys2_all = np.full((NCORES, NSLOT2), -1.0, dtype=np.float32)
    idx16_all = np.zeros((NCORES, NSLOT2), dtype=np.int16)
    keys2_all[s2_core, slot2] = s2_key
    idx16_all[s2_core, slot2] = (s2_src - s2_chnk * CH).astype(np.int16)

    keys2_pg = _to_bf16(
        keys2_all.reshape(NCORES, TOTAL_G2, 128).transpose(0, 2, 1).copy())
    # wrapped idx layout: element i at [i % 16, i // 16], replicated to 128
    idxw = idx16_all.reshape(NCORES, NSLOT2 // 16, 16).transpose(0, 2, 1)
    idxw = np.tile(idxw, (1, 8, 1)).copy()                   # [NC,128,NSLOT2/16]

    # scale columns for layer 2: col (w2, r*W2+o)
    sc2 = np.zeros((NCORES, NW2 * 4 * W2), dtype=np.float32)
    r_grid = np.repeat(np.arange(R), W2)
    for k in range(NCORES):
        nodes_w = (k * PC + (np.arange(NW2)[:, None] * W2
                             + np.tile(np.arange(W2), R)[None, :]))
        nodes_w = np.minimum(nodes_w, N - 1)
        sc2[k] = recip[r_grid[None, :], nodes_w].reshape(-1)
    sc2_bf = _to_bf16(sc2)
    sc2_rep = np.broadcast_to(
        sc2_bf[:, None, :], (NCORES, 128, NW2 * 4 * W2)).copy()

    # batch one-hot [PC, 64] per core
    bone = np.zeros((NCORES, PC, NGRAPH), dtype=np.float32)
    for k in range(NCORES):
        nd = k * PC + np.arange(PC)
        real = nd < N
        bone[k, real, batch[nd[real]]] = 1.0
    bone_bf = _to_bf16(bone)

    # x^T own block, padded feat rows [128, PC]
    xT = np.zeros((NCORES, 128, PC), dtype=np.float32)
    for k in range(NCORES):
        nd = k * PC + np.arange(PC)
        real = nd < N
        xT[k][:15][:, real] = x[nd[real]].T
    xT_bf = _to_bf16(xT)

    # weights (replicated)
    def padw(w, rows):
        out = np.zeros((128, H), dtype=np.float32)
        out[:rows] = w
        return _to_bf16(out)

    W1p = np.stack([padw(np.asarray(W1)[r], 15) for r in range(R)])
    root1p = padw(np.asarray(root1), 15)
    W2p = np.stack([padw(np.asarray(W2w)[r], H) for r in range(R)])
    root2p = padw(np.asarray(root2), H)
    b1f = np.asarray(b1, dtype=np.float32).reshape(H, 1)
    b2f = np.asarray(b2, dtype=np.float32).reshape(H, 1)

    in_maps = []
    for k in range(NCORES):
        in_maps.append({
            "xsl": xsl_pg[k],          # [128, TOTAL_G, F1] bf16
            "keys": keys_pg[k],        # [128, TOTAL_G] bf16
            "keys2": keys2_pg[k],      # [128, TOTAL_G2] bf16
            "idx2": idxw[k],           # [128, NSLOT2/16] int16
            "screp2": sc2_rep[k],      # [128, NW2*256] bf16
            "bone": bone_bf[k],        # [PC, 64] bf16
            "xT": xT_bf[k],            # [128, PC] bf16
            "W1p": W1p, "root1p": root1p,
            "W2p": W2p, "root2p": root2p,
            "b1": b1f, "b2": b2f,
        })

    gcounts = np.maximum(np.bincount(batch, minlength=NGRAPH), 1).astype(np.float32)
    host_ctx = {"cap": cap, "g_off": g_off, "TOTAL_G": TOTAL_G,
                "cap2": cap2, "group_base": group_base,
                "tile_g0": tile_g0, "seg_off": seg_off,
                "TOTAL_G2": TOTAL_G2, "gcounts": gcounts}
    return in_maps, host_ctx


# ------------------------------------------------------------- device build
def _build_nc(hc, legalize=True):
    cap, g_off, TOTAL_G = hc["cap"], hc["g_off"], hc["TOTAL_G"]
    cap2, group_base = hc["cap2"], hc["group_base"]
    tile_g0, seg_off, TOTAL_G2 = hc["tile_g0"], hc["seg_off"], hc["TOTAL_G2"]

    GT1 = max(int(g_off[(t + 1) * TW] - g_off[t * TW]) for t in range(NT))
    GT2 = max(int(tile_g0[t + 1] - tile_g0[t]) for t in range(NT))

    nc = Bacc("TRN2", num_devices=NCORES, num_swdge_queues=4)
    xsl = nc.dram_tensor("xsl", [128, TOTAL_G, F1], _bf16, kind="ExternalInput")
    keys = nc.dram_tensor("keys", [128, TOTAL_G], _bf16, kind="ExternalInput")
    keys2 = nc.dram_tensor("keys2", [128, TOTAL_G2], _bf16, kind="ExternalInput")
    idx2 = nc.dram_tensor("idx2", [128, TOTAL_G2 * 8], mybir.dt.int16,
                          kind="ExternalInput")
    screp2 = nc.dram_tensor("screp2", [128, NW2 * 4 * W2], _bf16,
                            kind="ExternalInput")
    bone = nc.dram_tensor("bone", [PC, NGRAPH], _bf16, kind="ExternalInput")
    xT = nc.dram_tensor("xT", [128, PC], _bf16, kind="ExternalInput")
    W1p = nc.dram_tensor("W1p", [R, 128, H], _bf16, kind="ExternalInput")
    root1p = nc.dram_tensor("root1p", [128, H], _bf16, kind="ExternalInput")
    W2p = nc.dram_tensor("W2p", [R, 128, H], _bf16, kind="ExternalInput")
    root2p = nc.dram_tensor("root2p", [128, H], _bf16, kind="ExternalInput")
    b1 = nc.dram_tensor("b1", [H, 1], _f32, kind="ExternalInput")
    b2 = nc.dram_tensor("b2", [H, 1], _f32, kind="ExternalInput")
    h1own = nc.dram_tensor("h1own", [PC, H], _bf16, kind="Internal")
    h1tab = nc.dram_tensor("h1tab", [NPAD, H], _bf16, kind="Internal",
                           addr_space="Shared")
    pool_out = nc.dram_tensor("pool_out", [NGRAPH, H], _f32, kind="ExternalOutput")

    with TileContext(nc, num_cores=NCORES) as tc:
        import contextlib
        with contextlib.ExitStack() as ctx:
            const_p = ctx.enter_context(tc.tile_pool(name="const", bufs=1))
            wpool = ctx.enter_context(tc.tile_pool(name="wts", bufs=1))
            hpool = ctx.enter_context(tc.tile_pool(name="hT", bufs=1))
            feed_p = ctx.enter_context(tc.tile_pool(name="feed", bufs=3))
            oh_p = ctx.enter_context(tc.tile_pool(name="oh", bufs=2))
            g_p = ctx.enter_context(tc.tile_pool(name="gat", bufs=2))
            idx_p = ctx.enter_context(tc.tile_pool(name="idxp", bufs=3))
            sb_p = ctx.enter_context(tc.tile_pool(name="stile", bufs=2))
            sc_p = ctx.enter_context(tc.tile_pool(name="sctile", bufs=2))
            tok_p = ctx.enter_context(tc.tile_pool(name="tok", bufs=3))
            ho_p = ctx.enter_context(tc.tile_pool(name="hop", bufs=2))
            bo_p = ctx.enter_context(tc.tile_pool(name="bo", bufs=3))
            misc_p = ctx.enter_context(tc.tile_pool(name="misc", bufs=2))
            ps_agg = ctx.enter_context(
                tc.tile_pool(name="ps_agg", bufs=2, space="PSUM"))
            ps_out = ctx.enter_context(
                tc.tile_pool(name="ps_out", bufs=1, space="PSUM"))
            ps_tr = ctx.enter_context(
                tc.tile_pool(name="ps_tr", bufs=2, space="PSUM"))
            ps_pool = ctx.enter_context(
                tc.tile_pool(name="ps_pool", bufs=1, space="PSUM"))

            nc.gpsimd.load_library(mlp)

            # iota constants (one period; broadcast via 0-step AP in is_equal)
            iota_i1 = const_p.tile([128, 128], mybir.dt.int32, tag="ioi1")
            nc.gpsimd.iota(iota_i1[:], pattern=[[1, 128]], base=0,
                           channel_multiplier=0)
            iota1 = const_p.tile([128, 128], _bf16, tag="io1")
            nc.vector.tensor_copy(iota1[:], iota_i1[:])
            iota_i2 = const_p.tile([128, 4 * W2], mybir.dt.int32, tag="ioi2")
            nc.gpsimd.iota(iota_i2[:], pattern=[[1, 4 * W2]], base=0,
                           channel_multiplier=0)
            iota2 = const_p.tile([128, 4 * W2], _bf16, tag="io2")
            nc.vector.tensor_copy(iota2[:], iota_i2[:])
            ident = const_p.tile([128, 128], _bf16, tag="ident")
            make_identity(nc, ident[:])

            # weights resident in SBUF
            w1t = [wpool.tile([128, H], _bf16, tag=f"w1_{r}", name=f"w1_{r}")
                   for r in range(R)]
            w2t = [wpool.tile([128, H], _bf16, tag=f"w2_{r}", name=f"w2_{r}")
                   for r in range(R)]
            r1t = wpool.tile([128, H], _bf16, tag="r1")
            r2t = wpool.tile([128, H], _bf16, tag="r2")
            b1t = wpool.tile([H, 1], _f32, tag="b1")
            b2t = wpool.tile([H, 1], _f32, tag="b2")
            for r in range(R):
                nc.sync.dma_start(out=w1t[r][:], in_=W1p[r])
                nc.sync.dma_start(out=w2t[r][:], in_=W2p[r])
            nc.sync.dma_start(out=r1t[:], in_=root1p[:, :])
            nc.sync.dma_start(out=r2t[:], in_=root2p[:, :])
            nc.sync.dma_start(out=b1t[:], in_=b1[:, :])
            nc.sync.dma_start(out=b2t[:], in_=b2[:, :])

            hT_x = hpool.tile([128, PC], _bf16, tag="hT_x")     # layer1 rhs
            hT_1 = hpool.tile([128, PC], _bf16, tag="hT_1")     # layer1 out
            nc.sync.dma_start(out=hT_x[:], in_=xT[:, :])

            pool_acc = const_p.tile([NGRAPH, H], _f32, tag="pacc")
            nc.vector.memset(pool_acc[:], 0.0)

            def bcast_inner(tile_ap, ncols, inner):
                """[128, ncols] -> AP [128, ncols, inner] (step-0 inner)."""
                base = tile_ap
                newap = [list(base.ap[0]), [base.ap[-1][0], ncols], [0, inner]]
                return AP(base.tensor, base.offset, newap)

            def bcast_outer(tile_ap, ng, inner):
                """[128, inner] -> AP [128, ng, inner] (step-0 middle)."""
                base = tile_ap
                newap = [list(base.ap[0]), [0, ng], [base.ap[-1][0], inner]]
                return AP(base.tensor, base.offset, newap)

            def tail(L, t, op_ps, hT_out_full):
                """bias+relu, transpose to token-major, h1own store / pool."""
                nsl = slice(t * 256, (t + 1) * 256)
                if hT_out_full is not None:
                    hO = hT_out_full[:, nsl]
                else:
                    hO = ho_p.tile([128, 256], _bf16, tag="hO", name="hO")
                nc.scalar.activation(
                    out=hO[:, :] if hT_out_full is None else hO,
                    in_=op_ps[:, :],
                    func=mybir.ActivationFunctionType.Relu,
                    bias=b1t[:] if L == 1 else b2t[:], scale=1.0)
                for half in range(2):
                    hsl = slice(half * 128, (half + 1) * 128)
                    tr = ps_tr.tile([128, 128], _bf16, tag="tr")
                    nc.tensor.transpose(tr[:, :], hO[:, hsl], ident[:])
                    tok = tok_p.tile([128, 128], _bf16, tag="tok")
                    nc.scalar.activation(
                        out=tok[:], in_=tr[:, :],
                        func=mybir.ActivationFunctionType.Copy)
                    row0 = t * 256 + half * 128
                    if L == 1:
                        nc.sync.dma_start(
                            out=h1own[row0:row0 + 128, :], in_=tok[:])
                    else:
                        bt_t = bo_p.tile([128, NGRAPH], _bf16, tag="bt")
                        nc.sync.dma_start(
                            out=bt_t[:], in_=bone[row0:row0 + 128, :])
                        pp = ps_pool.tile([NGRAPH, H], _f32, tag="pp")
                        nc.tensor.matmul(pp[:, :], lhsT=bt_t[:],
                                         rhs=tok[:], start=True, stop=True)
                        nc.vector.tensor_tensor(
                            out=pool_acc[:], in0=pool_acc[:], in1=pp[:, :],
                            op=mybir.AluOpType.add)

            # ---------------------------------------------------- layer 1
            for t in range(NT):
                w0 = t * TW
                g0, g1 = int(g_off[w0]), int(g_off[w0 + TW])
                ng = g1 - g0
                feedt = feed_p.tile([128, GT1, F1], _bf16, tag="feed")
                nc.sync.dma_start(out=feedt[:, :ng, :], in_=xsl[:, g0:g1, :])
                keyt = misc_p.tile([128, GT1], _bf16, tag="keyt")
                nc.sync.dma_start(out=keyt[:, :ng], in_=keys[:, g0:g1])
                oht = oh_p.tile([128, GT1 * 128], _fp8, tag="oht")
                nc.vector.tensor_tensor(
                    out=oht[:, :ng * 128],
                    in0=bcast_outer(iota1[:, :], ng, 128),
                    in1=bcast_inner(keyt[:, :ng], ng, 128),
                    op=mybir.AluOpType.is_equal)

                agg = [ps_agg.tile([128, 512], _f32, tag=f"agg{h}",
                                   name=f"agg{h}") for h in range(2)]
                for wi in range(TW):
                    w = w0 + wi
                    colsl = slice((wi % 4) * 128, (wi % 4) * 128 + 128)
                    ngw = int(cap[w])
                    for j in range(ngw):
                        gg = int(g_off[w]) + j - g0
                        nc.tensor.matmul(
                            agg[wi // 4][:F1, colsl],
                            lhsT=feedt[:, gg, :],
                            rhs=oht[:, gg * 128:(gg + 1) * 128],
                            start=(j == 0), stop=(j == ngw - 1))

                st = sb_p.tile([128, TW * 128], _bf16, tag="st")
                for h in range(2):
                    nc.vector.tensor_copy(
                        st[:F1, h * 512:(h + 1) * 512], agg[h][:F1, :])

                op_ps = ps_out.tile([128, 256], _f32, tag="ops")
                nsl = slice(t * 256, (t + 1) * 256)
                nc.tensor.matmul(op_ps[:, :], lhsT=r1t[:F1, :],
                                 rhs=hT_x[:F1, nsl], start=True, stop=False)
                st3 = st[:F1, :].rearrange("p (a b) -> p a b", b=128)
                for r in range(R):
                    nc.tensor.matmul(op_ps[:, :], lhsT=w1t[r][:F1, :],
                                     rhs=st3[:, :, r * W:(r + 1) * W],
                                     start=False, stop=(r == R - 1))
                tail(1, t, op_ps, hT_1)

            # ------------------------------------------------- all-gather
            ag = nc.gpsimd.collective_compute(
                "AllGather", mybir.AluOpType.bypass,
                replica_groups=[list(range(NCORES))],
                ins=[h1own[:, :]], outs=[h1tab[:, :]])
            ag_inst = ag.ins if hasattr(ag, "ins") else ag
            ag_dep_done = [False]
            qrr = [0]

            # ---------------------------------------------------- layer 2
            for t in range(NT):
                tg0, tg1 = int(tile_g0[t]), int(tile_g0[t + 1])
                gt_n = tg1 - tg0
                idxt = idx_p.tile([128, GT2 * 8], mybir.dt.int16, tag="idxt")
                nc.sync.dma_start(out=idxt[:, :gt_n * 8],
                                  in_=idx2[:, tg0 * 8:tg1 * 8])
                keyt2 = misc_p.tile([128, GT2], _bf16, tag="keyt2")
                nc.sync.dma_start(out=keyt2[:, :gt_n], in_=keys2[:, tg0:tg1])

                gtb = g_p.tile([128, GT2, H], _bf16, tag="gtb")
                for c in range(NCHUNK):
                    s0 = int(seg_off[t][c]) - tg0
                    s1 = int(seg_off[t][c + 1]) - tg0
                    for b0 in range(s0, s1, GCH2):
                        b1_ = min(b0 + GCH2, s1)
                        nb = b1_ - b0
                        gi = nc.gpsimd.dma_gather(
                            gtb[:, b0:b1_, :],
                            h1tab[c * CH:(c + 1) * CH, :],
                            idxt[:, b0 * 8:b1_ * 8],
                            nb * 128, nb * 128, H, queue_num=c)
                        if not ag_dep_done[0]:
                            add_dep_helper(
                                gi.ins, ag_inst,
                                reason="L2 gather reads AllGather output")
                            ag_dep_done[0] = True

                oht2 = oh_p.tile([128, GT2 * 4 * W2], _fp8, tag="oht2")
                nc.vector.tensor_tensor(
                    out=oht2[:, :gt_n * 4 * W2],
                    in0=bcast_outer(iota2[:, :], gt_n, 4 * W2),
                    in1=bcast_inner(keyt2[:, :gt_n], gt_n, 4 * W2),
                    op=mybir.AluOpType.is_equal)

                agg = [ps_agg.tile([128, 512], _f32, tag=f"agg{h}",
                                   name=f"agg{h}") for h in range(2)]
                for wi in range(TW2):
                    w = t * TW2 + wi
                    colsl = slice((wi % 2) * 256, (wi % 2) * 256 + 256)
                    seq = [(c, j) for c in range(NCHUNK)
                           for j in range(int(cap2[w, c]))]
                    for si, (c, j) in enumerate(seq):
                        gg = int(group_base[w, c]) + j - tg0
                        nc.tensor.matmul(
                            agg[wi // 2][:, colsl],
                            lhsT=gtb[:, gg, :],
                            rhs=oht2[:, gg * 256:(gg + 1) * 256],
                            start=(si == 0), stop=(si == len(seq) - 1))

                st = sb_p.tile([128, TW2 * 256], _bf16, tag="st")
                sct = sc_p.tile([128, TW2 * 256], _bf16, tag="sct")
                nc.sync.dma_start(
                    out=sct[:], in_=screp2[:, t * 1024:(t + 1) * 1024])
                for h in range(2):
                    nc.vector.tensor_tensor(
                        out=st[:, h * 512:(h + 1) * 512],
                        in0=agg[h][:, :], in1=sct[:, h * 512:(h + 1) * 512],
                        op=mybir.AluOpType.mult)

                op_ps = ps_out.tile([128, 256], _f32, tag="ops")
                nsl = slice(t * 256, (t + 1) * 256)
                nc.tensor.matmul(op_ps[:, :], lhsT=r2t[:, :],
                                 rhs=hT_1[:, nsl], start=True, stop=False)
                st3 = st[:, :].rearrange("p (a b) -> p a b", b=256)
                for r in range(R):
                    nc.tensor.matmul(op_ps[:, :], lhsT=w2t[r][:, :],
                                     rhs=st3[:, :, r * W2:(r + 1) * W2],
                                     start=False, stop=(r == R - 1))
                tail(2, t, op_ps, None)

            nc.sync.dma_start(out=pool_out[:, :], in_=pool_acc[:])

    nc.finalize()
    if legalize:
        _legalize_waits(nc)
    return nc


# ------------------------------------------------------------- runner
_CACHE = {}


def _get_compiled(hc):
    key = ("nc", hc["TOTAL_G"], hc["TOTAL_G2"],
           tuple(hc["cap"].tolist()),
           tuple(hc["cap2"].reshape(-1).tolist()))
    if key not in _CACHE:
        import jax
        from jax.sharding import Mesh, PartitionSpec
        from jax.experimental.shard_map import shard_map
        from concourse.bass2jax import (
            _bass_exec_p, partition_id_tensor, install_neuronx_cc_hook)
        install_neuronx_cc_hook()
        nc = _build_nc(hc)

        partition_name = (nc.partition_id_tensor.name
                          if nc.partition_id_tensor else None)
        in_names, out_names, out_avals = [], [], []
        for alloc in nc.m.functions[0].allocations:
            if not isinstance(alloc, mybir.MemoryLocationSet):
                continue
            name = alloc.memorylocations[0].name
            if alloc.kind == "ExternalInput":
                if name != partition_name and name != (
                        nc.dbg_addr.name if nc.dbg_addr is not None else None):
                    in_names.append(name)
            elif alloc.kind == "ExternalOutput":
                out_names.append(name)
                out_avals.append(jax.core.ShapedArray(
                    tuple(alloc.tensor_shape), mybir.dt.np(alloc.dtype)))
        n_params, n_outs = len(in_names), len(out_names)
        all_in = list(in_names) + list(out_names)
        if nc.dbg_addr is not None:
            all_in.append(nc.dbg_addr.name)
        if partition_name is not None:
            all_in.append(partition_name)

        def _body(*args):
            operands = list(args)
            if nc.dbg_addr is not None:
                operands.append(jax.numpy.zeros((1, 2), jax.numpy.uint32))
            if partition_name is not None:
                operands.append(partition_id_tensor())
            outs = _bass_exec_p.bind(
                *operands, out_avals=tuple(out_avals),
                in_names=tuple(all_in), out_names=tuple(out_names),
                lowering_input_output_aliases=(),
                sim_require_finite=False, sim_require_nnan=False, nc=nc)
            return tuple(outs)

        devices = jax.devices()[:NCORES]
        mesh = Mesh(np.asarray(devices), ("core",))
        sharded = jax.jit(
            shard_map(_body, mesh=mesh,
                      in_specs=(PartitionSpec("core"),) * (n_params + n_outs),
                      out_specs=(PartitionSpec("core"),) * n_outs,
                      check_rep=False),
            donate_argnums=tuple(range(n_params, n_params + n_outs)),
            keep_unused=True)
        _CACHE[key] = (sharded, in_names, out_names, out_avals, mesh)
    return _CACHE[key]


def run_device(in_maps, hc):
    import jax
    sharded, in_names, out_names, out_avals, mesh = _get_compiled(hc)
    concat_in = [
        np.concatenate([np.asarray(in_maps[c][name]) for c in range(NCORES)],
                       axis=0)
        for name in in_names]
    concat_zeros = [
        np.zeros((NCORES * a.shape[0], *a.shape[1:]), a.dtype)
        for a in out_avals]
    out_arrs = sharded(*concat_in, *concat_zeros)
    jax.block_until_ready(out_arrs)
    res = [
        {name: np.asarray(out_arrs[i]).reshape(NCORES, *out_avals[i].shape)[c]
         for i, name in enumerate(out_names)}
        for c in range(NCORES)]
    return res


def kernel(x, W1, root1, b1, W2, root2, b2, edge_index, edge_type, batch):
    in_maps, hc = _host_prep(x, W1, root1, b1, W2, root2, b2,
                             edge_index, edge_type, batch)
    res = run_device(in_maps, hc)
    total = np.zeros((NGRAPH, H), dtype=np.float32)
    for k in range(NCORES):
        total += res[k]["pool_out"]
    return (total / hc["gcounts"][:, None]).astype(np.float32)


# revision 17
# speedup vs baseline: 1.3372x; 1.1847x over previous
"""BasicRGCN Trainium2 kernel — 8-core SPMD Bass/Tile implementation.

Model (PyG-style RGCNConv x2 + global_mean_pool):
  h1 = relu(x @ root1 + b1 + sum_r mean_r(x_src) @ W1[r])
  h2 = relu(h1 @ root2 + b2 + sum_r mean_r(h1_src) @ W2[r])
  out[g] = mean over nodes in graph g of h2            -> [64, 128] f32

Distribution: nodes (and their incoming edges) are sharded over 8 cores by
destination id (12544 nodes/core). Per-relation weights are replicated.

Layer 1: edge features (15-dim x rows, scaled by 1/deg) are pre-gathered on
the host; aggregation uses a one-hot matmul over 32-node dst windows
(128 one-hot columns = 4 relations x 32 offsets).

Layer 2: h1 is exchanged with an AllGather into a replicated table, then
edge rows are fetched with batched SWDGE dma_gather instructions.  Since
gather indices are int16, the table is split into 4 chunks of 25088 rows;
edges are grouped into (64-node dst window, src chunk) cells so each
(tile, chunk) segment is one contiguous gather.  Aggregation uses 256-wide
one-hot matmuls (4 relations x 64 offsets).
"""
import sys
sys.path.insert(0, "/opt/trn_rl_repo")
import numpy as np

import concourse.bass as bass
import concourse.mybir as mybir
import concourse.tile as tile_mod
from concourse.tile import TileContext
from concourse.bacc import Bacc
from concourse.ap import AP
from concourse.masks import make_identity
from concourse.tile_rust import add_dep_helper
from concourse.library_config import mlp

# ---------------------------------------------------------------- constants
NCORES = 8
N = 100000
NPAD = 100352            # 8 * 12544
PC = NPAD // NCORES      # 12544 nodes per core
H = 128                  # hidden dim
F1 = 16                  # padded layer-1 input dim (15 real)
R = 4                    # relations
NGRAPH = 64
NT = 49                  # 256-node output tiles per core

# layer-1 aggregation structure
W = 32                   # dst window width (4W = 128 one-hot columns)
NW = PC // W             # 392 windows per core
TW = 8                   # windows per output tile (256 nodes)

# layer-2 aggregation structure
W2 = 64                  # dst window width (4*W2 = 256 one-hot columns)
NW2 = PC // W2           # 196 windows per core
TW2 = 4                  # windows per output tile (256 nodes)
NCHUNK = 4               # src chunks (int16 gather indices)
CH = NPAD // NCHUNK      # 25088 rows per chunk
GCH2 = 8                 # max groups (128 idx each) per dma_gather

_bf16 = mybir.dt.bfloat16
_f32 = mybir.dt.float32
_fp8 = mybir.dt.float8e4


def _to_bf16(a):
    import ml_dtypes
    return a.astype(ml_dtypes.bfloat16)


# ------------------------------------------------------- tile/walrus patches
def _patch_tile_drain():
    """This deployment's walrus accepts only ONE sync-wait per instruction:
    split the end-of-TileContext drain into single-wait drains."""
    def _patched(self, tick_clock, wait_clock):
        nc = self.nc
        drain_inst = nc.sync.drain()
        wait_clock.add_sem_waits(
            drain_inst.ins, tile_mod.ScopedClock({None: tick_clock.global_clock})
        )
        si = drain_inst.ins.sync_info
        if si is not None and si.on_wait and len(si.on_wait) > 1:
            waits = list(si.on_wait)
            si.on_wait = waits[:1]
            for i in range(1, len(waits)):
                extra = nc.sync.drain()
                esi = extra.ins.sync_info
                if esi is None:
                    extra.ins.sync_info = mybir.SyncInfo(
                        on_wait=[waits[i]], on_update=[])
                else:
                    esi.on_wait = [waits[i]]
        nc.all_engine_barrier()
        assert self.sems is not None
        popped = nc._tile_sem_poison_stack.pop()
        assert popped is self._sem_poison
        nc.clear_and_free_semaphores(list(self.sems.allocated().values()))
        nc.all_engine_barrier()
    TileContext._drain_and_barrier = _patched


_patch_tile_drain()
_legal_ctr = [0]


def _legalize_waits(nc, maxw=1):
    """Split >maxw sync-waits on any instruction onto preceding same-engine
    NoOps (engine streams are in-order, so this is semantics-preserving)."""
    for f in nc.m.functions:
        for blk in f.blocks:
            insts = list(blk.instructions)
            out = []
            changed = False
            for ins in insts:
                si = ins.sync_info
                if si is not None and si.on_wait and len(si.on_wait) > maxw:
                    waits = list(si.on_wait)
                    for i in range(0, len(waits) - maxw, maxw):
                        _legal_ctr[0] += 1
                        nop = mybir.InstNoOp(
                            name=f"legalw-{_legal_ctr[0]}", ins=[], outs=[])
                        nop.engine = ins.engine
                        nop.sync_info = mybir.SyncInfo(
                            on_wait=waits[i:i + maxw], on_update=[])
                        out.append(nop)
                    si.on_wait = waits[len(waits) - maxw:]
                    changed = True
                out.append(ins)
            if changed:
                blk.instructions = out


# ------------------------------------------------------------- host prep
def _host_prep(x, W1, root1, b1, W2w, root2, b2, edge_index, edge_type, batch):
    """Shard/repack all inputs. Returns (per_core_inmaps, host_ctx)."""
    src = np.asarray(edge_index[0], dtype=np.int64)
    dst = np.asarray(edge_index[1], dtype=np.int64)
    rel = np.asarray(edge_type, dtype=np.int64)
    batch = np.asarray(batch, dtype=np.int64)
    x = np.asarray(x, dtype=np.float32)
    E = src.shape[0]

    # per-(relation, dst) in-degree counts -> mean scale
    cnt = np.zeros((R, N), dtype=np.int64)
    np.add.at(cnt, (rel, dst), 1)
    recip = (1.0 / np.maximum(cnt, 1)).astype(np.float32)   # [R, N]

    core_of = dst // PC
    woff = dst % PC

    # ---------------- layer 1 (unchanged structure, host-gathered feed)
    win = woff // W
    key = rel * W + (woff % W)                               # [0, 4W)

    cw = np.zeros((NCORES, NW), dtype=np.int64)
    np.add.at(cw, (core_of, win), 1)
    cap = np.maximum(np.ceil(cw.max(axis=0) / 128).astype(np.int64), 1)  # [NW]
    g_off = np.concatenate([[0], np.cumsum(cap)])            # [NW+1]
    TOTAL_G = int(g_off[-1])

    order = np.lexsort((win, core_of))
    s_src, s_rel, s_dst = src[order], rel[order], dst[order]
    s_core, s_win = core_of[order], win[order]
    s_key = key[order]
    s_scale = recip[s_rel, s_dst].astype(np.float32)

    NSLOT = TOTAL_G * 128
    keys_all = np.full((NCORES, NSLOT), -1.0, dtype=np.float32)
    xsl_all = np.zeros((NCORES, NSLOT, F1), dtype=np.float32)

    cell_id = s_core * NW + s_win
    cell_start = np.zeros(NCORES * NW + 1, dtype=np.int64)
    np.add.at(cell_start, cell_id + 1, 1)
    cell_start = np.cumsum(cell_start)
    pos_in_cell = np.arange(E) - cell_start[cell_id]
    slot = (g_off[s_win] * 128 + pos_in_cell).astype(np.int64)

    keys_all[s_core, slot] = s_key
    xsl_all[s_core, slot, :15] = x[s_src] * s_scale[:, None]

    keys_pg = _to_bf16(
        keys_all.reshape(NCORES, TOTAL_G, 128).transpose(0, 2, 1).copy())
    xsl_pg = _to_bf16(
        xsl_all.reshape(NCORES, TOTAL_G, 128, F1).transpose(0, 2, 1, 3).copy())

    # ---------------- layer 2 (chunked dma_gather structure)
    win2 = woff // W2
    key2 = rel * W2 + (woff % W2)                            # [0, 4*W2)
    chnk = src // CH

    cnt2 = np.zeros((NCORES, NW2, NCHUNK), dtype=np.int64)
    np.add.at(cnt2, (core_of, win2, chnk), 1)
    cap2 = np.ceil(cnt2.max(axis=0) / 128).astype(np.int64)  # [NW2, NCHUNK]
    cap2[:, 0] = np.maximum(cap2[:, 0], 1)

    # group layout: per tile t, chunk-major segments of its TW2 windows
    group_base = np.zeros((NW2, NCHUNK), dtype=np.int64)
    tile_g0 = np.zeros(NT + 1, dtype=np.int64)
    seg_off = np.zeros((NT, NCHUNK + 1), dtype=np.int64)
    g = 0
    for t in range(NT):
        tile_g0[t] = g
        for c in range(NCHUNK):
            seg_off[t][c] = g
            for wi in range(TW2):
                w = t * TW2 + wi
                group_base[w, c] = g
                g += cap2[w, c]
        seg_off[t][NCHUNK] = g
    tile_g0[NT] = g
    TOTAL_G2 = int(g)

    order2 = np.lexsort((chnk, win2, core_of))
    s2_src, s2_rel = src[order2], rel[order2]
    s2_core, s2_win, s2_chnk = core_of[order2], win2[order2], chnk[order2]
    s2_key = key2[order2]

    cell_id2 = (s2_core * NW2 + s2_win) * NCHUNK + s2_chnk
    cell_start2 = np.zeros(NCORES * NW2 * NCHUNK + 1, dtype=np.int64)
    np.add.at(cell_start2, cell_id2 + 1, 1)
    cell_start2 = np.cumsum(cell_start2)
    pos2 = np.arange(E) - cell_start2[cell_id2]
    slot2 = ((group_base[s2_win, s2_chnk] + pos2 // 128) * 128
             + pos2 % 128).astype(np.int64)

    NSLOT2 = TOTAL_G2 * 128
    keys2_all = np.full((NCORES, NSLOT2), -1.0, dtype=np.float32)
    idx16_all = np.zeros((NCORES, NSLOT2), dtype=np.int16)
    keys2_all[s2_core, slot2] = s2_key
    idx16_all[s2_core, slot2] = (s2_src - s2_chnk * CH).astype(np.int16)

    keys2_pg = _to_bf16(
        keys2_all.reshape(NCORES, TOTAL_G2, 128).transpose(0, 2, 1).copy())
    # wrapped idx layout: element i at [i % 16, i // 16], replicated to 128
    idxw = idx16_all.reshape(NCORES, NSLOT2 // 16, 16).transpose(0, 2, 1)
    idxw = np.tile(idxw, (1, 8, 1)).copy()                   # [NC,128,NSLOT2/16]

    # scale columns for layer 2: col (w2, r*W2+o)
    sc2 = np.zeros((NCORES, NW2 * 4 * W2), dtype=np.float32)
    r_grid = np.repeat(np.arange(R), W2)
    for k in range(NCORES):
        nodes_w = (k * PC + (np.arange(NW2)[:, None] * W2
                             + np.tile(np.arange(W2), R)[None, :]))
        nodes_w = np.minimum(nodes_w, N - 1)
        sc2[k] = recip[r_grid[None, :], nodes_w].reshape(-1)
    sc2_bf = _to_bf16(sc2)
    sc2_rep = np.broadcast_to(
        sc2_bf[:, None, :], (NCORES, 128, NW2 * 4 * W2)).copy()

    # batch one-hot [PC, 64] per core
    bone = np.zeros((NCORES, PC, NGRAPH), dtype=np.float32)
    for k in range(NCORES):
        nd = k * PC + np.arange(PC)
        real = nd < N
        bone[k, real, batch[nd[real]]] = 1.0
    bone_bf = _to_bf16(bone)

    # x^T own block, padded feat rows [128, PC]
    xT = np.zeros((NCORES, 128, PC), dtype=np.float32)
    for k in range(NCORES):
        nd = k * PC + np.arange(PC)
        real = nd < N
        xT[k][:15][:, real] = x[nd[real]].T
    xT_bf = _to_bf16(xT)

    # weights (replicated)
    def padw(w, rows):
        out = np.zeros((128, H), dtype=np.float32)
        out[:rows] = w
        return _to_bf16(out)

    W1p = np.stack([padw(np.asarray(W1)[r], 15) for r in range(R)])
    root1p = padw(np.asarray(root1), 15)
    W2p = np.stack([padw(np.asarray(W2w)[r], H) for r in range(R)])
    root2p = padw(np.asarray(root2), H)
    b1f = np.asarray(b1, dtype=np.float32).reshape(H, 1)
    b2f = np.asarray(b2, dtype=np.float32).reshape(H, 1)

    in_maps = []
    for k in range(NCORES):
        in_maps.append({
            "xsl": xsl_pg[k],          # [128, TOTAL_G, F1] bf16
            "keys": keys_pg[k],        # [128, TOTAL_G] bf16
            "keys2": keys2_pg[k],      # [128, TOTAL_G2] bf16
            "idx2": idxw[k],           # [128, NSLOT2/16] int16
            "screp2": sc2_rep[k],      # [128, NW2*256] bf16
            "bone": bone_bf[k],        # [PC, 64] bf16
            "xT": xT_bf[k],            # [128, PC] bf16
            "W1p": W1p, "root1p": root1p,
            "W2p": W2p, "root2p": root2p,
            "b1": b1f, "b2": b2f,
        })

    gcounts = np.maximum(np.bincount(batch, minlength=NGRAPH), 1).astype(np.float32)
    host_ctx = {"cap": cap, "g_off": g_off, "TOTAL_G": TOTAL_G,
                "cap2": cap2, "group_base": group_base,
                "tile_g0": tile_g0, "seg_off": seg_off,
                "TOTAL_G2": TOTAL_G2, "gcounts": gcounts}
    return in_maps, host_ctx


# ------------------------------------------------------------- device build
def _build_nc(hc, legalize=True):
    cap, g_off, TOTAL_G = hc["cap"], hc["g_off"], hc["TOTAL_G"]
    cap2, group_base = hc["cap2"], hc["group_base"]
    tile_g0, seg_off, TOTAL_G2 = hc["tile_g0"], hc["seg_off"], hc["TOTAL_G2"]

    GT1 = max(int(g_off[(t + 1) * TW] - g_off[t * TW]) for t in range(NT))
    GT2 = max(int(tile_g0[t + 1] - tile_g0[t]) for t in range(NT))

    nc = Bacc("TRN2", num_devices=NCORES, num_swdge_queues=4)
    xsl = nc.dram_tensor("xsl", [128, TOTAL_G, F1], _bf16, kind="ExternalInput")
    keys = nc.dram_tensor("keys", [128, TOTAL_G], _bf16, kind="ExternalInput")
    keys2 = nc.dram_tensor("keys2", [128, TOTAL_G2], _bf16, kind="ExternalInput")
    idx2 = nc.dram_tensor("idx2", [128, TOTAL_G2 * 8], mybir.dt.int16,
                          kind="ExternalInput")
    screp2 = nc.dram_tensor("screp2", [128, NW2 * 4 * W2], _bf16,
                            kind="ExternalInput")
    bone = nc.dram_tensor("bone", [PC, NGRAPH], _bf16, kind="ExternalInput")
    xT = nc.dram_tensor("xT", [128, PC], _bf16, kind="ExternalInput")
    W1p = nc.dram_tensor("W1p", [R, 128, H], _bf16, kind="ExternalInput")
    root1p = nc.dram_tensor("root1p", [128, H], _bf16, kind="ExternalInput")
    W2p = nc.dram_tensor("W2p", [R, 128, H], _bf16, kind="ExternalInput")
    root2p = nc.dram_tensor("root2p", [128, H], _bf16, kind="ExternalInput")
    b1 = nc.dram_tensor("b1", [H, 1], _f32, kind="ExternalInput")
    b2 = nc.dram_tensor("b2", [H, 1], _f32, kind="ExternalInput")
    h1own = nc.dram_tensor("h1own", [PC, H], _bf16, kind="Internal")
    h1tab = nc.dram_tensor("h1tab", [NPAD, H], _bf16, kind="Internal",
                           addr_space="Shared")
    pool_out = nc.dram_tensor("pool_out", [NGRAPH, H], _f32, kind="ExternalOutput")

    with TileContext(nc, num_cores=NCORES) as tc:
        import contextlib
        with contextlib.ExitStack() as ctx:
            const_p = ctx.enter_context(tc.tile_pool(name="const", bufs=1))
            wpool = ctx.enter_context(tc.tile_pool(name="wts", bufs=1))
            hpool = ctx.enter_context(tc.tile_pool(name="hT", bufs=1))
            feed_p = ctx.enter_context(tc.tile_pool(name="feed", bufs=3))
            oh_p = ctx.enter_context(tc.tile_pool(name="oh", bufs=2))
            g_p = ctx.enter_context(tc.tile_pool(name="gat", bufs=3))
            idx_p = ctx.enter_context(tc.tile_pool(name="idxp", bufs=3))
            sb_p = ctx.enter_context(tc.tile_pool(name="stile", bufs=2))
            sc_p = ctx.enter_context(tc.tile_pool(name="sctile", bufs=2))
            tok_p = ctx.enter_context(tc.tile_pool(name="tok", bufs=3))
            ho_p = ctx.enter_context(tc.tile_pool(name="hop", bufs=2))
            bo_p = ctx.enter_context(tc.tile_pool(name="bo", bufs=3))
            misc_p = ctx.enter_context(tc.tile_pool(name="misc", bufs=2))
            ps_agg = ctx.enter_context(
                tc.tile_pool(name="ps_agg", bufs=2, space="PSUM"))
            ps_out = ctx.enter_context(
                tc.tile_pool(name="ps_out", bufs=1, space="PSUM"))
            ps_tr = ctx.enter_context(
                tc.tile_pool(name="ps_tr", bufs=2, space="PSUM"))
            ps_pool = ctx.enter_context(
                tc.tile_pool(name="ps_pool", bufs=1, space="PSUM"))

            nc.gpsimd.load_library(mlp)

            # iota constants (one period; broadcast via 0-step AP in is_equal)
            iota_i1 = const_p.tile([128, 128], mybir.dt.int32, tag="ioi1")
            nc.gpsimd.iota(iota_i1[:], pattern=[[1, 128]], base=0,
                           channel_multiplier=0)
            iota1 = const_p.tile([128, 128], _bf16, tag="io1")
            nc.vector.tensor_copy(iota1[:], iota_i1[:])
            iota_i2 = const_p.tile([128, 4 * W2], mybir.dt.int32, tag="ioi2")
            nc.gpsimd.iota(iota_i2[:], pattern=[[1, 4 * W2]], base=0,
                           channel_multiplier=0)
            iota2 = const_p.tile([128, 4 * W2], _bf16, tag="io2")
            nc.vector.tensor_copy(iota2[:], iota_i2[:])
            ident = const_p.tile([128, 128], _bf16, tag="ident")
            make_identity(nc, ident[:])

            # weights resident in SBUF
            w1t = [wpool.tile([128, H], _bf16, tag=f"w1_{r}", name=f"w1_{r}")
                   for r in range(R)]
            w2t = [wpool.tile([128, H], _bf16, tag=f"w2_{r}", name=f"w2_{r}")
                   for r in range(R)]
            r1t = wpool.tile([128, H], _bf16, tag="r1")
            r2t = wpool.tile([128, H], _bf16, tag="r2")
            b1t = wpool.tile([H, 1], _f32, tag="b1")
            b2t = wpool.tile([H, 1], _f32, tag="b2")
            for r in range(R):
                nc.sync.dma_start(out=w1t[r][:], in_=W1p[r])
                nc.sync.dma_start(out=w2t[r][:], in_=W2p[r])
            nc.sync.dma_start(out=r1t[:], in_=root1p[:, :])
            nc.sync.dma_start(out=r2t[:], in_=root2p[:, :])
            nc.sync.dma_start(out=b1t[:], in_=b1[:, :])
            nc.sync.dma_start(out=b2t[:], in_=b2[:, :])

            hT_x = hpool.tile([128, PC], _bf16, tag="hT_x")     # layer1 rhs
            hT_1 = hpool.tile([128, PC], _bf16, tag="hT_1")     # layer1 out
            nc.sync.dma_start(out=hT_x[:], in_=xT[:, :])

            pool_acc = const_p.tile([NGRAPH, H], _f32, tag="pacc")
            nc.vector.memset(pool_acc[:], 0.0)

            def bcast_inner(tile_ap, ncols, inner):
                """[128, ncols] -> AP [128, ncols, inner] (step-0 inner)."""
                base = tile_ap
                newap = [list(base.ap[0]), [base.ap[-1][0], ncols], [0, inner]]
                return AP(base.tensor, base.offset, newap)

            def bcast_outer(tile_ap, ng, inner):
                """[128, inner] -> AP [128, ng, inner] (step-0 middle)."""
                base = tile_ap
                newap = [list(base.ap[0]), [0, ng], [base.ap[-1][0], inner]]
                return AP(base.tensor, base.offset, newap)

            def tail(L, t, op_ps, hT_out_full):
                """bias+relu, transpose to token-major, h1own store / pool."""
                nsl = slice(t * 256, (t + 1) * 256)
                if hT_out_full is not None:
                    hO = hT_out_full[:, nsl]
                else:
                    hO = ho_p.tile([128, 256], _bf16, tag="hO", name="hO")
                nc.scalar.activation(
                    out=hO[:, :] if hT_out_full is None else hO,
                    in_=op_ps[:, :],
                    func=mybir.ActivationFunctionType.Relu,
                    bias=b1t[:] if L == 1 else b2t[:], scale=1.0)
                for half in range(2):
                    hsl = slice(half * 128, (half + 1) * 128)
                    tr = ps_tr.tile([128, 128], _bf16, tag="tr")
                    nc.tensor.transpose(tr[:, :], hO[:, hsl], ident[:])
                    tok = tok_p.tile([128, 128], _bf16, tag="tok")
                    nc.scalar.activation(
                        out=tok[:], in_=tr[:, :],
                        func=mybir.ActivationFunctionType.Copy)
                    row0 = t * 256 + half * 128
                    if L == 1:
                        nc.sync.dma_start(
                            out=h1own[row0:row0 + 128, :], in_=tok[:])
                    else:
                        bt_t = bo_p.tile([128, NGRAPH], _bf16, tag="bt")
                        nc.sync.dma_start(
                            out=bt_t[:], in_=bone[row0:row0 + 128, :])
                        pp = ps_pool.tile([NGRAPH, H], _f32, tag="pp")
                        nc.tensor.matmul(pp[:, :], lhsT=bt_t[:],
                                         rhs=tok[:], start=True, stop=True)
                        nc.vector.tensor_tensor(
                            out=pool_acc[:], in0=pool_acc[:], in1=pp[:, :],
                            op=mybir.AluOpType.add)

            # ---------------------------------------------------- layer 1
            for t in range(NT):
                w0 = t * TW
                g0, g1 = int(g_off[w0]), int(g_off[w0 + TW])
                ng = g1 - g0
                feedt = feed_p.tile([128, GT1, F1], _bf16, tag="feed")
                nc.sync.dma_start(out=feedt[:, :ng, :], in_=xsl[:, g0:g1, :])
                keyt = misc_p.tile([128, GT1], _bf16, tag="keyt")
                nc.sync.dma_start(out=keyt[:, :ng], in_=keys[:, g0:g1])
                oht = oh_p.tile([128, GT1 * 128], _fp8, tag="oht")
                nc.vector.tensor_tensor(
                    out=oht[:, :ng * 128],
                    in0=bcast_outer(iota1[:, :], ng, 128),
                    in1=bcast_inner(keyt[:, :ng], ng, 128),
                    op=mybir.AluOpType.is_equal)

                agg = [ps_agg.tile([128, 512], _f32, tag=f"agg{h}",
                                   name=f"agg{h}") for h in range(2)]
                for wi in range(TW):
                    w = w0 + wi
                    colsl = slice((wi % 4) * 128, (wi % 4) * 128 + 128)
                    ngw = int(cap[w])
                    for j in range(ngw):
                        gg = int(g_off[w]) + j - g0
                        nc.tensor.matmul(
                            agg[wi // 4][:F1, colsl],
                            lhsT=feedt[:, gg, :],
                            rhs=oht[:, gg * 128:(gg + 1) * 128],
                            start=(j == 0), stop=(j == ngw - 1))

                st = sb_p.tile([128, TW * 128], _bf16, tag="st")
                for h in range(2):
                    nc.vector.tensor_copy(
                        st[:F1, h * 512:(h + 1) * 512], agg[h][:F1, :])

                op_ps = ps_out.tile([128, 256], _f32, tag="ops")
                nsl = slice(t * 256, (t + 1) * 256)
                nc.tensor.matmul(op_ps[:, :], lhsT=r1t[:F1, :],
                                 rhs=hT_x[:F1, nsl], start=True, stop=False)
                st3 = st[:F1, :].rearrange("p (a b) -> p a b", b=128)
                for r in range(R):
                    nc.tensor.matmul(op_ps[:, :], lhsT=w1t[r][:F1, :],
                                     rhs=st3[:, :, r * W:(r + 1) * W],
                                     start=False, stop=(r == R - 1))
                tail(1, t, op_ps, hT_1)

            # ------------------------------------------------- all-gather
            ag = nc.gpsimd.collective_compute(
                "AllGather", mybir.AluOpType.bypass,
                replica_groups=[list(range(NCORES))],
                ins=[h1own[:, :]], outs=[h1tab[:, :]])
            ag_inst = ag.ins if hasattr(ag, "ins") else ag
            ag_dep_done = [False]
            qrr = [0]

            # ---------------------------------------------------- layer 2
            for t in range(NT):
                tg0, tg1 = int(tile_g0[t]), int(tile_g0[t + 1])
                gt_n = tg1 - tg0
                idxt = idx_p.tile([128, GT2 * 8], mybir.dt.int16, tag="idxt")
                nc.sync.dma_start(out=idxt[:, :gt_n * 8],
                                  in_=idx2[:, tg0 * 8:tg1 * 8])
                keyt2 = misc_p.tile([128, GT2], _bf16, tag="keyt2")
                nc.sync.dma_start(out=keyt2[:, :gt_n], in_=keys2[:, tg0:tg1])

                gtb = g_p.tile([128, GT2, H], _bf16, tag="gtb")
                # interleave queues round by round (chunk c stays on queue c)
                # to avoid head-of-line blocking at the in-order Pool SEQ
                blocks = []
                for c in range(NCHUNK):
                    s0 = int(seg_off[t][c]) - tg0
                    s1 = int(seg_off[t][c + 1]) - tg0
                    blocks.append([(c, b0, min(b0 + GCH2, s1))
                                   for b0 in range(s0, s1, GCH2)])
                rounds = max(len(b) for b in blocks)
                for rnd in range(rounds):
                    for c in range(NCHUNK):
                        if rnd >= len(blocks[c]):
                            continue
                        _, b0, b1_ = blocks[c][rnd]
                        nb = b1_ - b0
                        gi = nc.gpsimd.dma_gather(
                            gtb[:, b0:b1_, :],
                            h1tab[c * CH:(c + 1) * CH, :],
                            idxt[:, b0 * 8:b1_ * 8],
                            nb * 128, nb * 128, H, queue_num=c)
                        if not ag_dep_done[0]:
                            add_dep_helper(
                                gi.ins, ag_inst,
                                reason="L2 gather reads AllGather output")
                            ag_dep_done[0] = True

                oht2 = oh_p.tile([128, GT2 * 4 * W2], _fp8, tag="oht2")
                nc.vector.tensor_tensor(
                    out=oht2[:, :gt_n * 4 * W2],
                    in0=bcast_outer(iota2[:, :], gt_n, 4 * W2),
                    in1=bcast_inner(keyt2[:, :gt_n], gt_n, 4 * W2),
                    op=mybir.AluOpType.is_equal)

                agg = [ps_agg.tile([128, 512], _f32, tag=f"agg{h}",
                                   name=f"agg{h}") for h in range(2)]
                for wi in range(TW2):
                    w = t * TW2 + wi
                    colsl = slice((wi % 2) * 256, (wi % 2) * 256 + 256)
                    seq = [(c, j) for c in range(NCHUNK)
                           for j in range(int(cap2[w, c]))]
                    for si, (c, j) in enumerate(seq):
                        gg = int(group_base[w, c]) + j - tg0
                        nc.tensor.matmul(
                            agg[wi // 2][:, colsl],
                            lhsT=gtb[:, gg, :],
                            rhs=oht2[:, gg * 256:(gg + 1) * 256],
                            start=(si == 0), stop=(si == len(seq) - 1))

                st = sb_p.tile([128, TW2 * 256], _bf16, tag="st")
                sct = sc_p.tile([128, TW2 * 256], _bf16, tag="sct")
                nc.sync.dma_start(
                    out=sct[:], in_=screp2[:, t * 1024:(t + 1) * 1024])
                for h in range(2):
                    nc.vector.tensor_tensor(
                        out=st[:, h * 512:(h + 1) * 512],
                        in0=agg[h][:, :], in1=sct[:, h * 512:(h + 1) * 512],
                        op=mybir.AluOpType.mult)

                op_ps = ps_out.tile([128, 256], _f32, tag="ops")
                nsl = slice(t * 256, (t + 1) * 256)
                nc.tensor.matmul(op_ps[:, :], lhsT=r2t[:, :],
                                 rhs=hT_1[:, nsl], start=True, stop=False)
                st3 = st[:, :].rearrange("p (a b) -> p a b", b=256)
                for r in range(R):
                    nc.tensor.matmul(op_ps[:, :], lhsT=w2t[r][:, :],
                                     rhs=st3[:, :, r * W2:(r + 1) * W2],
                                     start=False, stop=(r == R - 1))
                tail(2, t, op_ps, None)

            nc.sync.dma_start(out=pool_out[:, :], in_=pool_acc[:])

    nc.finalize()
    if legalize:
        _legalize_waits(nc)
    return nc


# ------------------------------------------------------------- runner
_CACHE = {}


def _get_compiled(hc):
    key = ("nc", hc["TOTAL_G"], hc["TOTAL_G2"],
           tuple(hc["cap"].tolist()),
           tuple(hc["cap2"].reshape(-1).tolist()))
    if key not in _CACHE:
        import jax
        from jax.sharding import Mesh, PartitionSpec
        from jax.experimental.shard_map import shard_map
        from concourse.bass2jax import (
            _bass_exec_p, partition_id_tensor, install_neuronx_cc_hook)
        install_neuronx_cc_hook()
        nc = _build_nc(hc)

        partition_name = (nc.partition_id_tensor.name
                          if nc.partition_id_tensor else None)
        in_names, out_names, out_avals = [], [], []
        for alloc in nc.m.functions[0].allocations:
            if not isinstance(alloc, mybir.MemoryLocationSet):
                continue
            name = alloc.memorylocations[0].name
            if alloc.kind == "ExternalInput":
                if name != partition_name and name != (
                        nc.dbg_addr.name if nc.dbg_addr is not None else None):
                    in_names.append(name)
            elif alloc.kind == "ExternalOutput":
                out_names.append(name)
                out_avals.append(jax.core.ShapedArray(
                    tuple(alloc.tensor_shape), mybir.dt.np(alloc.dtype)))
        n_params, n_outs = len(in_names), len(out_names)
        all_in = list(in_names) + list(out_names)
        if nc.dbg_addr is not None:
            all_in.append(nc.dbg_addr.name)
        if partition_name is not None:
            all_in.append(partition_name)

        def _body(*args):
            operands = list(args)
            if nc.dbg_addr is not None:
                operands.append(jax.numpy.zeros((1, 2), jax.numpy.uint32))
            if partition_name is not None:
                operands.append(partition_id_tensor())
            outs = _bass_exec_p.bind(
                *operands, out_avals=tuple(out_avals),
                in_names=tuple(all_in), out_names=tuple(out_names),
                lowering_input_output_aliases=(),
                sim_require_finite=False, sim_require_nnan=False, nc=nc)
            return tuple(outs)

        devices = jax.devices()[:NCORES]
        mesh = Mesh(np.asarray(devices), ("core",))
        sharded = jax.jit(
            shard_map(_body, mesh=mesh,
                      in_specs=(PartitionSpec("core"),) * (n_params + n_outs),
                      out_specs=(PartitionSpec("core"),) * n_outs,
                      check_rep=False),
            donate_argnums=tuple(range(n_params, n_params + n_outs)),
            keep_unused=True)
        _CACHE[key] = (sharded, in_names, out_names, out_avals, mesh)
    return _CACHE[key]


def run_device(in_maps, hc):
    import jax
    sharded, in_names, out_names, out_avals, mesh = _get_compiled(hc)
    concat_in = [
        np.concatenate([np.asarray(in_maps[c][name]) for c in range(NCORES)],
                       axis=0)
        for name in in_names]
    concat_zeros = [
        np.zeros((NCORES * a.shape[0], *a.shape[1:]), a.dtype)
        for a in out_avals]
    out_arrs = sharded(*concat_in, *concat_zeros)
    jax.block_until_ready(out_arrs)
    res = [
        {name: np.asarray(out_arrs[i]).reshape(NCORES, *out_avals[i].shape)[c]
         for i, name in enumerate(out_names)}
        for c in range(NCORES)]
    return res


def kernel(x, W1, root1, b1, W2, root2, b2, edge_index, edge_type, batch):
    in_maps, hc = _host_prep(x, W1, root1, b1, W2, root2, b2,
                             edge_index, edge_type, batch)
    res = run_device(in_maps, hc)
    total = np.zeros((NGRAPH, H), dtype=np.float32)
    for k in range(NCORES):
        total += res[k]["pool_out"]
    return (total / hc["gcounts"][:, None]).astype(np.float32)
